# revision 3
# baseline (speedup 1.0000x reference)
"""DeepSeek-style MoE layer (64 routed experts, top-6 grouped routing, 2
shared experts) on 8 Trainium2 NeuronCores.

v2: identical slot-tile structure to the baseline, but routed expert
weights are stored/streamed as fp8-e3m4 (halving the dominant HBM
traffic) with global power-of-2 scales folded into the SiLU activation
scale (gate) and the y psum->sbuf copy (up*down).  Activations are fp16
(PE takes mixed fp16 x fp8 operands; accumulate is fp32 either way).
Output partials are fp16 (summed in fp32 on host).
"""
import numpy as np
import ml_dtypes

import concourse.bacc as bacc
import concourse.mybir as mybir
from concourse import tile
from concourse.bass_utils import run_bass_kernel_spmd

F16 = np.float16
BF16 = ml_dtypes.bfloat16
E3 = ml_dtypes.float8_e3m4
E3MAX = 15.5

T, H, E, I = 1024, 1024, 64, 512
NG, TKG, TOPK = 8, 3, 6
RSF = 2.5
P = 128
KT = H // P          # 8 k-tiles over hidden dim
IT = I // P          # 4 tiles over intermediate dim
HN = H // 512        # 2 output column chunks
TN = T // P          # 8 token tiles
N_CORES = 8
SI_SLICE = P         # shared-expert intermediate slice per core (2*512/8)


def _pow2floor(v):
    return 2.0 ** np.floor(np.log2(v))


# ---------------------------------------------------------------- routing
def _route(x, gate_w, e_bias):
    """Exact fp32 replica of the reference noaux_tc grouped top-k."""
    logits = x.astype(np.float32) @ gate_w.astype(np.float32)
    scores = 1.0 / (1.0 + np.exp(-logits))
    sc = scores + e_bias
    g = sc.reshape(T, NG, E // NG)
    top2 = np.sort(g, axis=-1)[:, :, -2:].sum(-1)
    gidx = np.argsort(-top2, axis=-1)[:, :TKG]
    gmask = np.zeros((T, NG), bool)
    gmask[np.arange(T)[:, None], gidx] = True
    emask = np.repeat(gmask, E // NG, axis=1)
    masked = np.where(emask, sc, -np.inf)
    ids = np.argsort(-masked, axis=-1)[:, :TOPK]
    w = np.take_along_axis(scores, ids, axis=1)
    w = w / w.sum(-1, keepdims=True)
    return ids, w


def _pack(ids, w):
    """Assign exactly E//N_CORES experts to each core (one weight stream per
    expert, no duplicate weight DMA).  Experts with load > P ("big", <= 2P)
    are placed at positions 6/7 of the per-core order; overflow tokens go to
    slot tiles 8/9 which structurally reuse the SBUF weights of positions
    6/7 (wsel below), keeping the module uniform across cores."""
    per_e = []
    for e in range(E):
        rows, cols = np.nonzero(ids == e)
        per_e.append((rows, w[rows, cols] * RSF))
    loads = np.array([len(r) for r, _ in per_e])
    epc = E // N_CORES                      # experts per core (8)
    if loads.max() <= 2 * P and (loads > P).sum() <= 2 * N_CORES:
        bigs = sorted([e for e in range(E) if loads[e] > P],
                      key=lambda e: -loads[e])
        smalls = sorted([e for e in range(E) if loads[e] <= P],
                        key=lambda e: -loads[e])
        core_exp = [[] for _ in range(N_CORES)]
        big_cnt = [0] * N_CORES
        slot_sum = [0] * N_CORES
        for e in bigs:
            c = min((i for i in range(N_CORES)
                     if big_cnt[i] < 2 and len(core_exp[i]) < epc),
                    key=lambda i: (big_cnt[i], slot_sum[i]))
            core_exp[c].append(e)
            big_cnt[c] += 1
            slot_sum[c] += loads[e]
        for e in smalls:
            c = min((i for i in range(N_CORES) if len(core_exp[i]) < epc),
                    key=lambda i: (slot_sum[i], len(core_exp[i])))
            core_exp[c].append(e)
            slot_sum[c] += loads[e]
        ns = epc + 2
        wsel = list(range(epc)) + [epc - 2, epc - 1]
        cores = []
        for c in range(N_CORES):
            exps = core_exp[c]
            order = [e for e in exps if loads[e] <= P] + \
                    [e for e in exps if loads[e] > P]
            order = order[:epc]
            tiles = []
            for s in range(epc):
                e = order[s]
                toks, wts = per_e[e]
                tiles.append((e, toks[:P], wts[:P]))
            for pos in (epc - 2, epc - 1):
                e = order[pos]
                toks, wts = per_e[e]
                tiles.append((e, toks[P:2 * P], wts[P:2 * P]))
            cores.append((order, tiles))
        return cores, ns, epc, wsel
    # fallback: generic tile bin-packing, one weight stream per tile
    tiles = []
    for e in range(E):
        rows, wts = per_e[e]
        for off in range(0, len(rows), P):
            tiles.append((e, rows[off:off + P], wts[off:off + P]))
    cores0 = [[] for _ in range(N_CORES)]
    for t in sorted(tiles, key=lambda z: -len(z[1])):
        c = min(range(N_CORES),
                key=lambda i: (len(cores0[i]), sum(len(z[1]) for z in cores0[i])))
        cores0[c].append(t)
    ns = max(len(c) for c in cores0)
    cores = []
    for c in range(N_CORES):
        ctiles = list(cores0[c])
        while len(ctiles) < ns:
            ctiles.append((0, np.zeros(0, np.int64), np.zeros(0, np.float32)))
        cores.append(([e for e, _, _ in ctiles], ctiles))
    return cores, ns, ns, list(range(ns))


def _prepare(inputs):
    """Host-side shard/dispatch: returns (in_maps, ns, n_wsets, wsel, scales)."""
    x = np.asarray(inputs["hidden_states"], np.float32)
    w_gate = np.asarray(inputs["w_gate"], np.float32)
    w_up = np.asarray(inputs["w_up"], np.float32)
    w_down = np.asarray(inputs["w_down"], np.float32)
    sw_gate = np.asarray(inputs["sw_gate"], np.float32)
    sw_up = np.asarray(inputs["sw_up"], np.float32)
    sw_down = np.asarray(inputs["sw_down"], np.float32)

    ids, w = _route(x, np.asarray(inputs["gate_w"], np.float32),
                    np.asarray(inputs["e_bias"], np.float32))
    cores, ns, n_wsets, wsel = _pack(ids, w)

    # global power-of-two fp8 scales per weight type (uniform across cores
    # so the SPMD module's baked-in descale constants are identical)
    SG = _pow2floor(0.75 * E3MAX / np.abs(w_gate).max())
    SU = _pow2floor(0.75 * E3MAX / np.abs(w_up).max())
    SD = _pow2floor(0.75 * E3MAX / np.abs(w_down).max())

    # xt layout [P, KT*T]: xt[p, k*T + t] = x[t, k*P + p]
    xt = np.ascontiguousarray(
        x.T.reshape(KT, P, T).transpose(1, 0, 2).reshape(P, KT * T)).astype(BF16)
    # pre-reshaped expert weights in sbuf layout, fp8-e3m4 with global scales
    wg_sb_all = np.ascontiguousarray(
        (w_gate * SG).reshape(E, KT, P, I).transpose(0, 2, 1, 3).reshape(E, P, KT * I)).astype(E3)
    wu_sb_all = np.ascontiguousarray(
        (w_up * SU).reshape(E, KT, P, I).transpose(0, 2, 1, 3).reshape(E, P, KT * I)).astype(E3)
    wd_sb_all = np.ascontiguousarray(
        (w_down * SD).reshape(E, IT, P, H).transpose(0, 2, 1, 3).reshape(E, P, IT * H)).astype(E3)

    in_maps = []
    for c in range(N_CORES):
        order, ctiles = cores[c]
        xg = np.zeros((KT, P, ns * P), np.float32)
        st = np.zeros((ns, P, T), F16)
        wg_t = np.zeros((n_wsets, P, KT * I), E3)
        wu_t = np.zeros((n_wsets, P, KT * I), E3)
        wd_t = np.zeros((n_wsets, P, IT * H), E3)
        for j in range(n_wsets):
            e = order[j] if j < len(order) else 0
            wg_t[j] = wg_sb_all[e]
            wu_t[j] = wu_sb_all[e]
            wd_t[j] = wd_sb_all[e]
        for s, (e, toks, wts) in enumerate(ctiles):
            n = len(toks)
            if n:
                xg[:, :, s * P:s * P + n] = x[toks].T.reshape(KT, P, n)
                st[s, np.arange(n), toks] = wts.astype(F16)
        # xg layout [P, KT*ns*P]: xg2[p, k*ns*P + col] = x[tok(col), k*P + p]
        xg2 = np.ascontiguousarray(
            xg.transpose(1, 0, 2).reshape(P, KT * ns * P)).astype(F16)
        # st layout [P, ns*T]
        st2 = np.ascontiguousarray(
            st.transpose(1, 0, 2).reshape(P, ns * T))
        sl = slice(c * SI_SLICE, (c + 1) * SI_SLICE)
        swg = np.ascontiguousarray(
            sw_gate[:, sl].reshape(KT, P, SI_SLICE).transpose(1, 0, 2)
            .reshape(P, KT * SI_SLICE)).astype(BF16)
        swu = np.ascontiguousarray(
            sw_up[:, sl].reshape(KT, P, SI_SLICE).transpose(1, 0, 2)
            .reshape(P, KT * SI_SLICE)).astype(BF16)
        in_maps.append({
            "xg": xg2,
            "st": st2,
            "wg": wg_t,
            "wu": wu_t,
            "wd": wd_t,
            "xt": xt,
            "swg": swg,
            "swu": swu,
            "swd": sw_down[sl, :].astype(BF16),
            "ident": np.eye(P, dtype=BF16),
        })
    return in_maps, ns, n_wsets, wsel, (SG, SU, SD)


# ----------------------------------------------------------------- device
def _build(ns, n_wsets, wsel, scales, loop_n=1, skip_compute=False,
           skip_combine=False):
    """loop_n > 1 wraps the whole body in a device-side loop; used only for
    timing measurements (marginal cost per iteration = true exec time)."""
    import contextlib
    SG, SU, SD = scales
    FP16 = mybir.dt.float16
    E3D = mybir.dt.float8e3
    F32 = mybir.dt.float32
    SILU = mybir.ActivationFunctionType.Silu
    COPY = mybir.ActivationFunctionType.Copy
    BF = mybir.dt.bfloat16

    nc = bacc.Bacc(None, target_bir_lowering=False)
    xg_d = nc.declare_dram_parameter("xg", [P, KT * ns * P], FP16, isOutput=False)
    st_d = nc.declare_dram_parameter("st", [P, ns * T], FP16, isOutput=False)
    wg_d = nc.declare_dram_parameter("wg", [n_wsets, P, KT * I], E3D, isOutput=False)
    wu_d = nc.declare_dram_parameter("wu", [n_wsets, P, KT * I], E3D, isOutput=False)
    wd_d = nc.declare_dram_parameter("wd", [n_wsets, P, IT * H], E3D, isOutput=False)
    xt_d = nc.declare_dram_parameter("xt", [P, KT * T], BF, isOutput=False)
    swg_d = nc.declare_dram_parameter("swg", [P, KT * SI_SLICE], BF, isOutput=False)
    swu_d = nc.declare_dram_parameter("swu", [P, KT * SI_SLICE], BF, isOutput=False)
    swd_d = nc.declare_dram_parameter("swd", [SI_SLICE, H], BF, isOutput=False)
    ident_d = nc.declare_dram_parameter("ident", [P, P], BF, isOutput=False)
    out_d = nc.declare_dram_parameter("out", [T, H], BF, isOutput=True)

    with tile.TileContext(nc) as tc:
        with tc.tile_pool(name="big", bufs=1) as big, \
             tc.tile_pool(name="dbl", bufs=2) as dbl, \
             tc.tile_pool(name="wpool", bufs=4) as wpool, \
             tc.tile_pool(name="hpool", bufs=3) as hpool, \
             tc.tile_pool(name="opool", bufs=3) as opool, \
             tc.tile_pool(name="gup", bufs=4, space="PSUM") as gup, \
             tc.tile_pool(name="ypsum", bufs=2, space="PSUM") as ypsum, \
             tc.tile_pool(name="capool", bufs=2, space="PSUM") as ca, \
             tc.tile_pool(name="oapool", bufs=16) as oapool, \
             (tc.For_i(0, loop_n, 1) if loop_n > 1 else contextlib.nullcontext()):

            xt_sb = big.tile([P, KT * T], BF, tag="xt")
            nc.sync.dma_start(out=xt_sb[:], in_=xt_d[:])
            swg_sb = big.tile([P, KT * SI_SLICE], BF, tag="swg")
            nc.sync.dma_start(out=swg_sb[:], in_=swg_d[:])
            swu_sb = big.tile([P, KT * SI_SLICE], BF, tag="swu")
            nc.sync.dma_start(out=swu_sb[:], in_=swu_d[:])
            swd_sb = big.tile([P, H], BF, tag="swd")
            nc.sync.dma_start(out=swd_sb[:], in_=swd_d[:])
            ident_sb = big.tile([P, P], BF, tag="ident")
            nc.sync.dma_start(out=ident_sb[:], in_=ident_d[:])
            xg_sb = big.tile([P, KT * ns * P], FP16, tag="xg")
            nc.sync.dma_start(out=xg_sb[:], in_=xg_d[:])
            st_sb = dbl.tile([P, ns * T], FP16, tag="st")
            nc.sync.dma_start(out=st_sb[:], in_=st_d[:])
            y_sb = dbl.tile([P, ns * H], FP16, tag="y")
            hsh_sb = big.tile([P, T], BF, tag="hsh")

            # ---- shared experts (TP slice of SI on this core), all fp16
            for tn in range(T // 512 if not skip_compute else 0):
                pg = gup.tile([P, 512], F32, tag="pg")
                for k in range(KT):
                    nc.tensor.matmul(
                        pg[:],
                        swg_sb[:, k * SI_SLICE:(k + 1) * SI_SLICE],
                        xt_sb[:, k * T + tn * 512: k * T + tn * 512 + 512],
                        start=(k == 0), stop=(k == KT - 1))
                hg = hpool.tile([P, 512], F32, tag="hg")
                nc.scalar.activation(hg[:], pg[:], SILU)
                pu = gup.tile([P, 512], F32, tag="pg")
                for k in range(KT):
                    nc.tensor.matmul(
                        pu[:],
                        swu_sb[:, k * SI_SLICE:(k + 1) * SI_SLICE],
                        xt_sb[:, k * T + tn * 512: k * T + tn * 512 + 512],
                        start=(k == 0), stop=(k == KT - 1))
                nc.vector.tensor_mul(hsh_sb[:, tn * 512:(tn + 1) * 512], hg[:], pu[:])

            # ---- routed experts, one slot tile (<=128 tokens, one expert) at
            # a time; tiles with wsel[s] < s reuse already-resident weights
            wtiles = {}
            SPLIT = ns // 2
            oa_tiles = {}
            for s in range(ns):
                j = wsel[s]
                if j not in wtiles:
                    wgs = wpool.tile([P, KT * I], E3D, tag="wg")
                    nc.sync.dma_start(out=wgs[:], in_=wg_d[j])
                    wus = wpool.tile([P, KT * I], E3D, tag="wu")
                    nc.sync.dma_start(out=wus[:], in_=wu_d[j])
                    wds = wpool.tile([P, IT * H], E3D, tag="wd")
                    nc.sync.dma_start(out=wds[:], in_=wd_d[j])
                    wtiles[j] = (wgs, wus, wds)
                else:
                    wgs, wus, wds = wtiles[j]
                if skip_compute:
                    continue

                # gate/up with xg (tokens) stationary, fp8 weights moving
                pg = gup.tile([P, I], F32, tag="pg")
                for k in range(KT):
                    nc.tensor.matmul(
                        pg[:],
                        xg_sb[:, (k * ns + s) * P: (k * ns + s + 1) * P],
                        wgs[:, k * I: (k + 1) * I],
                        start=(k == 0), stop=(k == KT - 1))
                hg = hpool.tile([P, I], F32, tag="hg")
                nc.scalar.activation(hg[:], pg[:], SILU, scale=float(1.0 / SG))
                pu = gup.tile([P, I], F32, tag="pg")
                for k in range(KT):
                    nc.tensor.matmul(
                        pu[:],
                        xg_sb[:, (k * ns + s) * P: (k * ns + s + 1) * P],
                        wus[:, k * I: (k + 1) * I],
                        start=(k == 0), stop=(k == KT - 1))
                hb = hpool.tile([P, I], BF, tag="hb")
                nc.vector.tensor_mul(hb[:], hg[:], pu[:])  # carries SU factor
                pt = ypsum.tile([P, I], BF, tag="py")
                for isl in range(IT):
                    nc.tensor.transpose(
                        pt[:, isl * P:(isl + 1) * P],
                        hb[:, isl * P:(isl + 1) * P],
                        ident_sb[:])
                hbT = hpool.tile([P, I], BF, tag="hbT")
                nc.vector.tensor_copy(out=hbT[:], in_=pt[:])
                for hn in range(HN):
                    py = ypsum.tile([P, 512], F32, tag="py")
                    for isl in range(IT):
                        nc.tensor.matmul(
                            py[:],
                            hbT[:, isl * P:(isl + 1) * P],
                            wds[:, isl * H + hn * 512: isl * H + hn * 512 + 512],
                            start=(isl == 0), stop=(isl == IT - 1))
                    # descale (up*down fp8 scales) on the psum->sbuf copy
                    # (ACT engine: DVE psum-read + fp16-write faults the HW)
                    nc.scalar.activation(
                        y_sb[:, s * H + hn * 512: s * H + hn * 512 + 512],
                        py[:], COPY, scale=float(1.0 / (SU * SD)))
                if s == SPLIT - 1 and not (skip_compute or skip_combine):
                    # group-A combine over tiles 0..SPLIT-1
                    for tm in range(TN):
                        for hn in range(HN):
                            pa = ca.tile([P, 512], F32, tag="ca")
                            for s2 in range(SPLIT):
                                nc.tensor.matmul(
                                    pa[:],
                                    st_sb[:, s2 * T + tm * P: s2 * T + (tm + 1) * P],
                                    y_sb[:, s2 * H + hn * 512: s2 * H + hn * 512 + 512],
                                    start=(s2 == 0), stop=(s2 == SPLIT - 1))
                            oa = oapool.tile([P, 512], FP16, tag="oa")
                            nc.scalar.activation(oa[:], pa[:], COPY)
                            oa_tiles[(tm, hn)] = oa

            # ---- combine: out[T,H] = sum_s ST_s.T @ Y_s  + hsh.T @ swd
            if skip_compute or skip_combine:
                for tm in range(TN):
                    for hn in range(HN):
                        ob = opool.tile([P, 512], BF, tag="ob")
                        nc.any.memset(ob[:], 0.0)
                        nc.sync.dma_start(
                            out=out_d[tm * P:(tm + 1) * P, hn * 512:(hn + 1) * 512],
                            in_=ob[:])
            else:
              for tm in range(TN):
                for hn in range(HN):
                    pc = ca.tile([P, 512], F32, tag="ca")
                    for s in range(SPLIT, ns):
                        nc.tensor.matmul(
                            pc[:],
                            st_sb[:, s * T + tm * P: s * T + (tm + 1) * P],
                            y_sb[:, s * H + hn * 512: s * H + hn * 512 + 512],
                            start=(s == SPLIT), stop=False)
                    nc.tensor.matmul(
                        pc[:],
                        hsh_sb[:, tm * P:(tm + 1) * P],
                        swd_sb[:, hn * 512:(hn + 1) * 512],
                        start=False, stop=True)
                    ob = opool.tile([P, 512], BF, tag="ob")
                    nc.vector.tensor_tensor(
                        out=ob[:], in0=pc[:], in1=oa_tiles[(tm, hn)][:],
                        op=mybir.AluOpType.add)
                    nc.sync.dma_start(
                        out=out_d[tm * P:(tm + 1) * P, hn * 512:(hn + 1) * 512],
                        in_=ob[:])

    nc.finalize()
    return nc


def _run(nc, in_maps):
    res = run_bass_kernel_spmd(nc, in_maps, core_ids=list(range(N_CORES)))
    out = np.zeros((T, H), np.float32)
    for r in res.results:
        out += r["out"].astype(np.float32)
    return out


def kernel(**inputs):
    in_maps, ns, n_wsets, wsel, scales = _prepare(inputs)
    nc = _build(ns, n_wsets, wsel, scales)
    return _run(nc, in_maps)


# revision 5
# speedup vs baseline: 1.0119x; 1.0119x over previous
"""DeepSeek-style MoE layer (64 routed experts, top-6 grouped routing, 2
shared experts) on 8 Trainium2 NeuronCores.

v2: identical slot-tile structure to the baseline, but routed expert
weights are stored/streamed as fp8-e3m4 (halving the dominant HBM
traffic) with global power-of-2 scales folded into the SiLU activation
scale (gate) and the y psum->sbuf copy (up*down).  Activations are fp16
(PE takes mixed fp16 x fp8 operands; accumulate is fp32 either way).
Output partials are fp16 (summed in fp32 on host).
"""
import numpy as np
import ml_dtypes

import concourse.bacc as bacc
import concourse.mybir as mybir
from concourse import tile
from concourse.bass_utils import run_bass_kernel_spmd

F16 = np.float16
BF16 = ml_dtypes.bfloat16
E3 = ml_dtypes.float8_e3m4
E3MAX = 15.5

T, H, E, I = 1024, 1024, 64, 512
NG, TKG, TOPK = 8, 3, 6
RSF = 2.5
P = 128
KT = H // P          # 8 k-tiles over hidden dim
IT = I // P          # 4 tiles over intermediate dim
HN = H // 512        # 2 output column chunks
TN = T // P          # 8 token tiles
N_CORES = 8
SI_SLICE = P         # shared-expert intermediate slice per core (2*512/8)


def _pow2floor(v):
    return 2.0 ** np.floor(np.log2(v))


# ---------------------------------------------------------------- routing
def _route(x, gate_w, e_bias):
    """Exact fp32 replica of the reference noaux_tc grouped top-k."""
    logits = x.astype(np.float32) @ gate_w.astype(np.float32)
    scores = 1.0 / (1.0 + np.exp(-logits))
    sc = scores + e_bias
    g = sc.reshape(T, NG, E // NG)
    top2 = np.sort(g, axis=-1)[:, :, -2:].sum(-1)
    gidx = np.argsort(-top2, axis=-1)[:, :TKG]
    gmask = np.zeros((T, NG), bool)
    gmask[np.arange(T)[:, None], gidx] = True
    emask = np.repeat(gmask, E // NG, axis=1)
    masked = np.where(emask, sc, -np.inf)
    ids = np.argsort(-masked, axis=-1)[:, :TOPK]
    w = np.take_along_axis(scores, ids, axis=1)
    w = w / w.sum(-1, keepdims=True)
    return ids, w


def _pack(ids, w):
    """Assign exactly E//N_CORES experts to each core (one weight stream per
    expert, no duplicate weight DMA).  Experts with load > P ("big", <= 2P)
    are placed at positions 6/7 of the per-core order; overflow tokens go to
    slot tiles 8/9 which structurally reuse the SBUF weights of positions
    6/7 (wsel below), keeping the module uniform across cores."""
    per_e = []
    for e in range(E):
        rows, cols = np.nonzero(ids == e)
        per_e.append((rows, w[rows, cols] * RSF))
    loads = np.array([len(r) for r, _ in per_e])
    epc = E // N_CORES                      # experts per core (8)
    if loads.max() <= 2 * P and (loads > P).sum() <= 2 * N_CORES:
        bigs = sorted([e for e in range(E) if loads[e] > P],
                      key=lambda e: -loads[e])
        smalls = sorted([e for e in range(E) if loads[e] <= P],
                        key=lambda e: -loads[e])
        core_exp = [[] for _ in range(N_CORES)]
        big_cnt = [0] * N_CORES
        slot_sum = [0] * N_CORES
        for e in bigs:
            c = min((i for i in range(N_CORES)
                     if big_cnt[i] < 2 and len(core_exp[i]) < epc),
                    key=lambda i: (big_cnt[i], slot_sum[i]))
            core_exp[c].append(e)
            big_cnt[c] += 1
            slot_sum[c] += loads[e]
        for e in smalls:
            c = min((i for i in range(N_CORES) if len(core_exp[i]) < epc),
                    key=lambda i: (slot_sum[i], len(core_exp[i])))
            core_exp[c].append(e)
            slot_sum[c] += loads[e]
        ns = epc + 2
        wsel = list(range(epc)) + [epc - 2, epc - 1]
        cores = []
        for c in range(N_CORES):
            exps = core_exp[c]
            order = [e for e in exps if loads[e] <= P] + \
                    [e for e in exps if loads[e] > P]
            order = order[:epc]
            tiles = []
            for s in range(epc):
                e = order[s]
                toks, wts = per_e[e]
                tiles.append((e, toks[:P], wts[:P]))
            for pos in (epc - 2, epc - 1):
                e = order[pos]
                toks, wts = per_e[e]
                tiles.append((e, toks[P:2 * P], wts[P:2 * P]))
            cores.append((order, tiles))
        return cores, ns, epc, wsel
    # fallback: generic tile bin-packing, one weight stream per tile
    tiles = []
    for e in range(E):
        rows, wts = per_e[e]
        for off in range(0, len(rows), P):
            tiles.append((e, rows[off:off + P], wts[off:off + P]))
    cores0 = [[] for _ in range(N_CORES)]
    for t in sorted(tiles, key=lambda z: -len(z[1])):
        c = min(range(N_CORES),
                key=lambda i: (len(cores0[i]), sum(len(z[1]) for z in cores0[i])))
        cores0[c].append(t)
    ns = max(len(c) for c in cores0)
    cores = []
    for c in range(N_CORES):
        ctiles = list(cores0[c])
        while len(ctiles) < ns:
            ctiles.append((0, np.zeros(0, np.int64), np.zeros(0, np.float32)))
        cores.append(([e for e, _, _ in ctiles], ctiles))
    return cores, ns, ns, list(range(ns))


def _prepare(inputs):
    """Host-side shard/dispatch: returns (in_maps, ns, n_wsets, wsel, scales)."""
    x = np.asarray(inputs["hidden_states"], np.float32)
    w_gate = np.asarray(inputs["w_gate"], np.float32)
    w_up = np.asarray(inputs["w_up"], np.float32)
    w_down = np.asarray(inputs["w_down"], np.float32)
    sw_gate = np.asarray(inputs["sw_gate"], np.float32)
    sw_up = np.asarray(inputs["sw_up"], np.float32)
    sw_down = np.asarray(inputs["sw_down"], np.float32)

    ids, w = _route(x, np.asarray(inputs["gate_w"], np.float32),
                    np.asarray(inputs["e_bias"], np.float32))
    cores, ns, n_wsets, wsel = _pack(ids, w)

    # global power-of-two fp8 scales per weight type (uniform across cores
    # so the SPMD module's baked-in descale constants are identical)
    SG = _pow2floor(0.75 * E3MAX / np.abs(w_gate).max())
    SU = _pow2floor(0.75 * E3MAX / np.abs(w_up).max())
    SD = _pow2floor(0.75 * E3MAX / np.abs(w_down).max())

    # xt layout [P, KT*T]: xt[p, k*T + t] = x[t, k*P + p]
    xt = np.ascontiguousarray(
        x.T.reshape(KT, P, T).transpose(1, 0, 2).reshape(P, KT * T)).astype(BF16)
    # pre-reshaped expert weights in sbuf layout, fp8-e3m4 with global scales
    wg_sb_all = np.ascontiguousarray(
        (w_gate * SG).reshape(E, KT, P, I).transpose(0, 2, 1, 3).reshape(E, P, KT * I)).astype(E3)
    wu_sb_all = np.ascontiguousarray(
        (w_up * SU).reshape(E, KT, P, I).transpose(0, 2, 1, 3).reshape(E, P, KT * I)).astype(E3)
    wd_sb_all = np.ascontiguousarray(
        (w_down * SD).reshape(E, IT, P, H).transpose(0, 2, 1, 3).reshape(E, P, IT * H)).astype(E3)

    in_maps = []
    for c in range(N_CORES):
        order, ctiles = cores[c]
        xg = np.zeros((KT, P, ns * P), np.float32)
        st = np.zeros((ns, P, T), F16)
        wg_t = np.zeros((n_wsets, P, KT * I), E3)
        wu_t = np.zeros((n_wsets, P, KT * I), E3)
        wd_t = np.zeros((n_wsets, P, IT * H), E3)
        for j in range(n_wsets):
            e = order[j] if j < len(order) else 0
            wg_t[j] = wg_sb_all[e]
            wu_t[j] = wu_sb_all[e]
            wd_t[j] = wd_sb_all[e]
        for s, (e, toks, wts) in enumerate(ctiles):
            n = len(toks)
            if n:
                xg[:, :, s * P:s * P + n] = x[toks].T.reshape(KT, P, n)
                st[s, np.arange(n), toks] = wts.astype(F16)
        # xg layout [P, KT*ns*P]: xg2[p, k*ns*P + col] = x[tok(col), k*P + p]
        xg2 = np.ascontiguousarray(
            xg.transpose(1, 0, 2).reshape(P, KT * ns * P)).astype(F16)
        # st layout [P, ns*T]
        st2 = np.ascontiguousarray(
            st.transpose(1, 0, 2).reshape(P, ns * T))
        sl = slice(c * SI_SLICE, (c + 1) * SI_SLICE)
        swg = np.ascontiguousarray(
            sw_gate[:, sl].reshape(KT, P, SI_SLICE).transpose(1, 0, 2)
            .reshape(P, KT * SI_SLICE)).astype(BF16)
        swu = np.ascontiguousarray(
            sw_up[:, sl].reshape(KT, P, SI_SLICE).transpose(1, 0, 2)
            .reshape(P, KT * SI_SLICE)).astype(BF16)
        in_maps.append({
            "xg": xg2,
            "st": st2,
            "wg": wg_t,
            "wu": wu_t,
            "wd": wd_t,
            "xt": xt,
            "swg": swg,
            "swu": swu,
            "swd": sw_down[sl, :].astype(BF16),
            "ident": np.eye(P, dtype=BF16),
        })
    return in_maps, ns, n_wsets, wsel, (SG, SU, SD)


# ----------------------------------------------------------------- device
def _build(ns, n_wsets, wsel, scales, loop_n=1, skip_compute=False,
           skip_combine=False):
    """loop_n > 1 wraps the whole body in a device-side loop; used only for
    timing measurements (marginal cost per iteration = true exec time)."""
    import contextlib
    SG, SU, SD = scales
    FP16 = mybir.dt.float16
    E3D = mybir.dt.float8e3
    F32 = mybir.dt.float32
    SILU = mybir.ActivationFunctionType.Silu
    COPY = mybir.ActivationFunctionType.Copy
    BF = mybir.dt.bfloat16

    nc = bacc.Bacc(None, target_bir_lowering=False)
    xg_d = nc.declare_dram_parameter("xg", [P, KT * ns * P], FP16, isOutput=False)
    st_d = nc.declare_dram_parameter("st", [P, ns * T], FP16, isOutput=False)
    wg_d = nc.declare_dram_parameter("wg", [n_wsets, P, KT * I], E3D, isOutput=False)
    wu_d = nc.declare_dram_parameter("wu", [n_wsets, P, KT * I], E3D, isOutput=False)
    wd_d = nc.declare_dram_parameter("wd", [n_wsets, P, IT * H], E3D, isOutput=False)
    xt_d = nc.declare_dram_parameter("xt", [P, KT * T], BF, isOutput=False)
    swg_d = nc.declare_dram_parameter("swg", [P, KT * SI_SLICE], BF, isOutput=False)
    swu_d = nc.declare_dram_parameter("swu", [P, KT * SI_SLICE], BF, isOutput=False)
    swd_d = nc.declare_dram_parameter("swd", [SI_SLICE, H], BF, isOutput=False)
    ident_d = nc.declare_dram_parameter("ident", [P, P], BF, isOutput=False)
    out_d = nc.declare_dram_parameter("out", [T, H], BF, isOutput=True)

    with tile.TileContext(nc) as tc:
        with tc.tile_pool(name="big", bufs=1) as big, \
             tc.tile_pool(name="dbl", bufs=2) as dbl, \
             tc.tile_pool(name="wpool", bufs=4) as wpool, \
             tc.tile_pool(name="hpool", bufs=3) as hpool, \
             tc.tile_pool(name="opool", bufs=6) as opool, \
             tc.tile_pool(name="gup", bufs=3, space="PSUM") as gup, \
             tc.tile_pool(name="ypsum", bufs=2, space="PSUM") as ypsum, \
             tc.tile_pool(name="capool", bufs=3, space="PSUM") as ca, \
             tc.tile_pool(name="oapool", bufs=16) as oapool, \
             (tc.For_i(0, loop_n, 1) if loop_n > 1 else contextlib.nullcontext()):

            xt_sb = big.tile([P, KT * T], BF, tag="xt")
            nc.sync.dma_start(out=xt_sb[:], in_=xt_d[:])
            swg_sb = big.tile([P, KT * SI_SLICE], BF, tag="swg")
            nc.sync.dma_start(out=swg_sb[:], in_=swg_d[:])
            swu_sb = big.tile([P, KT * SI_SLICE], BF, tag="swu")
            nc.sync.dma_start(out=swu_sb[:], in_=swu_d[:])
            swd_sb = big.tile([P, H], BF, tag="swd")
            nc.sync.dma_start(out=swd_sb[:], in_=swd_d[:])
            ident_sb = big.tile([P, P], BF, tag="ident")
            nc.sync.dma_start(out=ident_sb[:], in_=ident_d[:])
            xg_sb = big.tile([P, KT * ns * P], FP16, tag="xg")
            nc.sync.dma_start(out=xg_sb[:], in_=xg_d[:])
            st_sb = dbl.tile([P, ns * T], FP16, tag="st")
            nc.sync.dma_start(out=st_sb[:], in_=st_d[:])
            y_sb = dbl.tile([P, ns * H], FP16, tag="y")
            hsh_sb = big.tile([P, T], BF, tag="hsh")

            # ---- shared experts (TP slice of SI on this core), all fp16
            for tn in range(T // 512 if not skip_compute else 0):
                pg = gup.tile([P, 512], F32, tag="pg")
                for k in range(KT):
                    nc.tensor.matmul(
                        pg[:],
                        swg_sb[:, k * SI_SLICE:(k + 1) * SI_SLICE],
                        xt_sb[:, k * T + tn * 512: k * T + tn * 512 + 512],
                        start=(k == 0), stop=(k == KT - 1))
                hg = hpool.tile([P, 512], F32, tag="hg")
                nc.scalar.activation(hg[:], pg[:], SILU)
                pu = gup.tile([P, 512], F32, tag="pg")
                for k in range(KT):
                    nc.tensor.matmul(
                        pu[:],
                        swu_sb[:, k * SI_SLICE:(k + 1) * SI_SLICE],
                        xt_sb[:, k * T + tn * 512: k * T + tn * 512 + 512],
                        start=(k == 0), stop=(k == KT - 1))
                nc.vector.tensor_mul(hsh_sb[:, tn * 512:(tn + 1) * 512], hg[:], pu[:])

            # ---- routed experts, one slot tile (<=128 tokens, one expert) at
            # a time; tiles with wsel[s] < s reuse already-resident weights
            wtiles = {}
            SPLIT = ns // 2
            oa_tiles = {}
            for s in range(ns):
                j = wsel[s]
                if j not in wtiles:
                    wgs = wpool.tile([P, KT * I], E3D, tag="wg")
                    nc.sync.dma_start(out=wgs[:], in_=wg_d[j])
                    wus = wpool.tile([P, KT * I], E3D, tag="wu")
                    nc.sync.dma_start(out=wus[:], in_=wu_d[j])
                    wds = wpool.tile([P, IT * H], E3D, tag="wd")
                    nc.sync.dma_start(out=wds[:], in_=wd_d[j])
                    wtiles[j] = (wgs, wus, wds)
                else:
                    wgs, wus, wds = wtiles[j]
                if skip_compute:
                    continue

                # gate/up with xg (tokens) stationary, fp8 weights moving
                pg = gup.tile([P, I], F32, tag="pg")
                for k in range(KT):
                    nc.tensor.matmul(
                        pg[:],
                        xg_sb[:, (k * ns + s) * P: (k * ns + s + 1) * P],
                        wgs[:, k * I: (k + 1) * I],
                        start=(k == 0), stop=(k == KT - 1))
                hg = hpool.tile([P, I], F32, tag="hg")
                nc.scalar.activation(hg[:], pg[:], SILU, scale=float(1.0 / SG))
                pu = gup.tile([P, I], F32, tag="pg")
                for k in range(KT):
                    nc.tensor.matmul(
                        pu[:],
                        xg_sb[:, (k * ns + s) * P: (k * ns + s + 1) * P],
                        wus[:, k * I: (k + 1) * I],
                        start=(k == 0), stop=(k == KT - 1))
                hb = hpool.tile([P, I], BF, tag="hb")
                nc.vector.tensor_mul(hb[:], hg[:], pu[:])  # carries SU factor
                pt = ypsum.tile([P, I], BF, tag="py")
                for isl in range(IT):
                    nc.tensor.transpose(
                        pt[:, isl * P:(isl + 1) * P],
                        hb[:, isl * P:(isl + 1) * P],
                        ident_sb[:])
                hbT = hpool.tile([P, I], BF, tag="hbT")
                nc.vector.tensor_copy(out=hbT[:], in_=pt[:])
                for hn in range(HN):
                    py = ypsum.tile([P, 512], F32, tag="py")
                    for isl in range(IT):
                        nc.tensor.matmul(
                            py[:],
                            hbT[:, isl * P:(isl + 1) * P],
                            wds[:, isl * H + hn * 512: isl * H + hn * 512 + 512],
                            start=(isl == 0), stop=(isl == IT - 1))
                    # descale (up*down fp8 scales) on the psum->sbuf copy
                    # (ACT engine: DVE psum-read + fp16-write faults the HW)
                    nc.scalar.activation(
                        y_sb[:, s * H + hn * 512: s * H + hn * 512 + 512],
                        py[:], COPY, scale=float(1.0 / (SU * SD)))
                if s == SPLIT - 1 and not (skip_compute or skip_combine):
                    # group-A combine over tiles 0..SPLIT-1
                    for tm in range(TN):
                        for hn in range(HN):
                            pa = ca.tile([P, 512], F32, tag="ca")
                            for s2 in range(SPLIT):
                                nc.tensor.matmul(
                                    pa[:],
                                    st_sb[:, s2 * T + tm * P: s2 * T + (tm + 1) * P],
                                    y_sb[:, s2 * H + hn * 512: s2 * H + hn * 512 + 512],
                                    start=(s2 == 0), stop=(s2 == SPLIT - 1))
                            oa = oapool.tile([P, 512], FP16, tag="oa")
                            nc.scalar.activation(oa[:], pa[:], COPY)
                            oa_tiles[(tm, hn)] = oa

            # ---- combine: out[T,H] = sum_s ST_s.T @ Y_s  + hsh.T @ swd
            if skip_compute or skip_combine:
                for tm in range(TN):
                    for hn in range(HN):
                        ob = opool.tile([P, 512], BF, tag="ob")
                        nc.any.memset(ob[:], 0.0)
                        nc.sync.dma_start(
                            out=out_d[tm * P:(tm + 1) * P, hn * 512:(hn + 1) * 512],
                            in_=ob[:])
            else:
              for tm in range(TN):
                for hn in range(HN):
                    pc = ca.tile([P, 512], F32, tag="ca")
                    for s in range(SPLIT, ns):
                        nc.tensor.matmul(
                            pc[:],
                            st_sb[:, s * T + tm * P: s * T + (tm + 1) * P],
                            y_sb[:, s * H + hn * 512: s * H + hn * 512 + 512],
                            start=(s == SPLIT), stop=False)
                    nc.tensor.matmul(
                        pc[:],
                        hsh_sb[:, tm * P:(tm + 1) * P],
                        swd_sb[:, hn * 512:(hn + 1) * 512],
                        start=False, stop=True)
                    ob = opool.tile([P, 512], BF, tag="ob")
                    nc.vector.tensor_tensor(
                        out=ob[:], in0=pc[:], in1=oa_tiles[(tm, hn)][:],
                        op=mybir.AluOpType.add)
                    nc.sync.dma_start(
                        out=out_d[tm * P:(tm + 1) * P, hn * 512:(hn + 1) * 512],
                        in_=ob[:])

    nc.finalize()
    return nc


def _run(nc, in_maps):
    res = run_bass_kernel_spmd(nc, in_maps, core_ids=list(range(N_CORES)))
    out = np.zeros((T, H), np.float32)
    for r in res.results:
        out += r["out"].astype(np.float32)
    return out


def kernel(**inputs):
    in_maps, ns, n_wsets, wsel, scales = _prepare(inputs)
    nc = _build(ns, n_wsets, wsel, scales)
    return _run(nc, in_maps)
